# revision 9
# baseline (speedup 1.0000x reference)
"""AliasFreeActivation Trainium2 kernel (v5: fp8-DoubleRow down path, batched
PSUM evacuation, GpSimd bias-add).

out = crop10(down2(leaky_relu(up4(x + bias)) * sqrt2))   [4,256,236,236]

Decomposition per (batch,channel) image (1024 images, 128 per core):
  leaky_relu(t)*s = 0.6*s*t + 0.4*s*|t|   (slope 0.2)
so with y = up4(xb):
  out = Down(0.4*sqrt2*|y|)  +  Down(0.6*sqrt2*y)
The linear term collapses through composed matrices Mv = A@D.

Stages (contraction on SBUF partitions; image stationary so outputs chain):
  s1  v1[w,ho]   = sum_h xb[h,w] A[h,ho]        1 MM  N=512  fp16
  sA  u1[w,hd]   = sum_h xb[h,w] Mv[h,hd]       1 MM  N=256  fp16
  s2  p2[ho,wo]  = sum_w v1[w,ho] A2[w,wo]      4 MM  N=512  fp16
  abs Y = |p2| -> fp8                           (2 batched DVE/ACT passes)
  s3  z[wo,hd]   = sum_ho Y[ho,wo] D[ho,hd]     8 MM  fp8 DoubleRow banded
  s4  o[hd,wd]   = sum_wo z[wo,hd] D[wo,wd]     8 MM  fp16 banded
  sB  o += sum_w u1[w,hd] Mh[w,wd]              2 MM  N=236  fp16

PSUM (8 banks): big 2x[128,2,512] (4 banks) holding {P1|PA} {m0|m1} {m2|m3},
p3 2x[128,2,256] (2) holding s3 chunk pairs, p4 2x[128,2,256] (2) for the two
output row-blocks.  Sub-bank slot sharing relies on hardware lazy
zero-on-first-write after a single start=True per bank.
U1 and the four Z chunks live in one [128,5,236] fp16 tensor so each PSUM
tile pair drains with a single batched instruction.
"""
import numpy as np

UP, DOWN, MARGIN, NEG_SLOPE = 4, 2, 10, 0.2
SQRT2 = 1.4142135623730951
H = W = 128
OUT = 236
NCORES = 8
NIMG = 128

# s4 down-matrix window per 128-row K-chunk, cropped to [10, 246)
DWIN4 = [(10, 67), (61, 131), (125, 195), (189, 246)]
# s3 DoubleRow pair windows: pair j covers ho chunks (2j, 2j+1), union bands
DWIN3 = [(10, 131), (125, 246)]

_cache = {}


def _build_nc(nimg=NIMG):
    import concourse.bacc as bacc
    import concourse.bass as bass
    import concourse.tile as tile
    from concourse import mybir

    F32 = mybir.dt.float32
    F16 = mybir.dt.float16
    F8 = mybir.dt.float8e4
    AF = mybir.ActivationFunctionType
    ALU = mybir.AluOpType
    DR = mybir.MatmulPerfMode.DoubleRow

    nc = bacc.Bacc("TRN2", target_bir_lowering=False)
    x_d = nc.dram_tensor("x", [nimg, H, W], F32, kind="ExternalInput")
    b_d = nc.dram_tensor("bias", [nimg], F32, kind="ExternalInput")
    c_d = nc.dram_tensor("cm", [128, 1796], F16, kind="ExternalInput")
    c8_d = nc.dram_tensor("cd", [128, 484], F8, kind="ExternalInput")
    o_d = nc.dram_tensor("out", [nimg, OUT, OUT], F32, kind="ExternalOutput")

    with tile.TileContext(nc) as tc:
        with (
            tc.tile_pool(name="const", bufs=1) as const,
            tc.tile_pool(name="xin", bufs=4) as xin,
            tc.tile_pool(name="xbp", bufs=3) as xbp,
            tc.tile_pool(name="v1p", bufs=2) as v1p,
            tc.tile_pool(name="yp", bufs=2) as yp,
            tc.tile_pool(name="zup", bufs=2) as zup,
            tc.tile_pool(name="op", bufs=3) as op_,
            tc.tile_pool(name="big", bufs=2, space="PSUM") as bigp,
            tc.tile_pool(name="misc", bufs=3, space="PSUM") as mscp,
            tc.tile_pool(name="p4", bufs=1, space="PSUM") as p4p,
        ):
            cm = const.tile([128, 1796], F16)
            nc.sync.dma_start(out=cm, in_=c_d[:])
            cd = const.tile([128, 4, 121], F8)
            nc.sync.dma_start(out=cd, in_=c8_d[:])
            A_sb = cm[:, 0:512]
            A2_sb = cm[:, 512:1024]                    # 0.4*sqrt2*A
            dw0 = 1024

            def D4_sb(k):
                o0, o1 = DWIN4[k]
                return cm[:, dw0 + 70 * k: dw0 + 70 * k + (o1 - o0)]

            mv0 = dw0 + 280                            # Mv [128,256]
            Mv_sb = cm[:, mv0: mv0 + 256]
            mh0 = mv0 + 256                            # Mh [128,236] (cropped)
            Mh_sb = cm[:, mh0: mh0 + 236]

            bb = const.tile([128, nimg], F32)
            nc.gpsimd.dma_start(
                out=bb,
                in_=bass.AP(tensor=b_d[:].tensor, offset=0,
                            ap=[[0, 128], [1, nimg]]),
            )

            # warm the PE clock behind the const DMA
            pwarm = p4p.tile([128, 2, 256], F32, name="p4")
            nc.tensor.matmul(out=pwarm[:32, 0, :], lhsT=cm[:, :32],
                             rhs=cm[:, :256], start=True, stop=True)

            for i in range(nimg):
                X = xin.tile([128, W], F32)
                nc.sync.dma_start(out=X, in_=x_d[i])
                Xb = xbp.tile([128, W], F16)
                # bias add + fp16 cast on GpSimd: slow (~2us) but fully
                # off the critical path once the pipeline is deep enough
                nc.gpsimd.tensor_scalar(Xb, X, bb[:, i:i + 1], None, ALU.add)

                V1 = v1p.tile([128, 512], F16)
                Y = yp.tile([128, 4, 512], F8)
                ZU = zup.tile([128, 5, 236], F16)   # slot0=U1, 1+m=Z chunks

                # --- s1 and sA into 1-bank misc-pool tiles -------------
                P1 = mscp.tile([128, 512], F32, name="ms")
                nc.tensor.matmul(out=P1, lhsT=Xb, rhs=A_sb,
                                 start=True, stop=True)
                nc.vector.tensor_copy(out=V1[:, 0:128], in_=P1[:, 0:128])
                nc.vector.tensor_copy(out=V1[:, 128:512], in_=P1[:, 128:512])
                PA = mscp.tile([128, 256], F32, name="ms")
                nc.tensor.matmul(out=PA, lhsT=Xb, rhs=Mv_sb,
                                 start=True, stop=True)
                nc.vector.tensor_copy(out=ZU[:, 0, :],
                                      in_=PA[:, MARGIN:MARGIN + OUT])

                # --- tiles B,C: s2 + batched abs -> fp8 Y --------------
                for half in range(2):
                    tb = bigp.tile([128, 2, 512], F32, name="tb")
                    for s in range(2):
                        m = 2 * half + s
                        nc.tensor.matmul(out=tb[:, s, :],
                                         lhsT=V1[:, 128 * m:128 * (m + 1)],
                                         rhs=A2_sb, start=True, stop=True)
                        if half == 1:
                            # solo abs per chunk: completes sooner, so s3
                            # (which needs all of Y) starts earlier
                            nc.scalar.activation(out=Y[:, m, :],
                                                 in_=tb[:, s, :],
                                                 func=AF.Abs, bias=0.0,
                                                 scale=1.0)
                    if half == 0:
                        nc.scalar.activation(out=Y[:, 0:2, :],
                                             in_=tb[:, :, :],
                                             func=AF.Abs, bias=0.0, scale=1.0)

                # --- s3: fp8 DoubleRow banded, pairs of wo chunks ------
                for half in range(2):
                    t3 = mscp.tile([128, 2, 256], F32, name="ms")
                    for s in range(2):
                        m = 2 * half + s
                        for j in range(2):
                            o0, o1 = DWIN3[j]
                            nc.tensor.matmul(
                                out=t3[:, s, o0:o1],
                                lhsT=Y[:, 2 * j:2 * j + 2,
                                       128 * m:128 * (m + 1)],
                                rhs=cd[:, 2 * j:2 * j + 2, :o1 - o0],
                                start=(s == 0 and j == 0), stop=(j == 1),
                                perf_mode=DR, skip_group_check=True)
                    a = 1 + 2 * half
                    nc.vector.tensor_copy(
                        out=ZU[:, a:a + 2, :],
                        in_=t3[:, :, MARGIN:MARGIN + OUT])

                # --- s4 + sB: banded down horizontal + linear term -----
                P4 = p4p.tile([128, 2, 256], F32, name="p4")
                for mo, (h0, h1) in enumerate(((0, 128), (128, OUT))):
                    rows = h1 - h0
                    for k in (0, 1):
                        o0, o1 = DWIN4[k]
                        nc.tensor.matmul(
                            out=P4[:rows, mo, o0:o1],
                            lhsT=ZU[:, 1 + k, h0:h1],
                            rhs=D4_sb(k), start=(mo == 0 and k == 0),
                            stop=False, skip_group_check=True)
                    nc.tensor.matmul(
                        out=P4[:rows, mo, MARGIN:MARGIN + OUT],
                        lhsT=ZU[:, 0, h0:h1],
                        rhs=Mh_sb, start=False, stop=False,
                        skip_group_check=True)
                    for k in (2, 3):
                        o0, o1 = DWIN4[k]
                        nc.tensor.matmul(
                            out=P4[:rows, mo, o0:o1],
                            lhsT=ZU[:, 1 + k, h0:h1],
                            rhs=D4_sb(k), start=False,
                            stop=(mo == 1 and k == 3), skip_group_check=True)
                O = op_.tile([128, 2, OUT], F32)
                nc.scalar.activation(out=O, in_=P4[:, :, MARGIN:MARGIN + OUT],
                                     func=AF.Copy, bias=0.0, scale=1.0)
                nc.sync.dma_start(out=o_d[i, 0:128, :], in_=O[:, 0, :])
                nc.sync.dma_start(out=o_d[i, 128:OUT, :], in_=O[:OUT - 128, 1, :])

    nc.finalize()
    return nc


def _filter_matrices(up_filter, down_filter):
    fu = np.asarray(up_filter, dtype=np.float64)
    fd = np.asarray(down_filter, dtype=np.float64)
    i = np.arange(128)[:, None]
    o = np.arange(512)[None, :]
    t = 10 + o - 4 * i
    A = np.where((t >= 0) & (t < 24), fu[np.clip(t, 0, 23)], 0.0)
    s = np.arange(512)[:, None]
    o2 = np.arange(256)[None, :]
    t2 = 6 + 2 * o2 - s
    D = np.where((t2 >= 0) & (t2 < 12), fd[np.clip(t2, 0, 11)], 0.0)
    return A, D


def _pack_consts(up_filter, down_filter):
    import ml_dtypes
    A, D = _filter_matrices(up_filter, down_filter)
    cm = np.zeros((128, 1796), dtype=np.float16)
    cm[:, 0:512] = A.astype(np.float16)
    cm[:, 512:1024] = (A * (0.4 * SQRT2)).astype(np.float16)
    dw0 = 1024
    for k, (o0, o1) in enumerate(DWIN4):
        cm[:, dw0 + 70 * k: dw0 + 70 * k + (o1 - o0)] = \
            D[128 * k:128 * (k + 1), o0:o1].astype(np.float16)
    Mv = A @ D
    mv0 = dw0 + 280
    cm[:, mv0: mv0 + 256] = Mv.astype(np.float16)
    mh0 = mv0 + 256
    cm[:, mh0: mh0 + 236] = (Mv * (0.6 * SQRT2))[:, 10:246].astype(np.float16)

    cd = np.zeros((128, 4, 121), dtype=ml_dtypes.float8_e4m3)
    for j, (o0, o1) in enumerate(DWIN3):
        for sub in range(2):
            k = 2 * j + sub
            cd[:, 2 * j + sub, :o1 - o0] = \
                D[128 * k:128 * (k + 1), o0:o1].astype(ml_dtypes.float8_e4m3)
    return cm, cd.reshape(128, 484)


def _run(x, bias, up_filter, down_filter, trace=False):
    from concourse.bass_utils import run_bass_kernel_spmd

    if "nc" not in _cache:
        _cache["nc"] = _build_nc()
    nc = _cache["nc"]

    cm, cd = _pack_consts(up_filter, down_filter)
    xf = np.ascontiguousarray(np.asarray(x, dtype=np.float32)
                              .reshape(NCORES * NIMG, H, W))
    bias = np.asarray(bias, dtype=np.float32)
    bias_full = np.tile(bias, (NCORES * NIMG) // bias.shape[0])

    in_maps = []
    for c in range(NCORES):
        in_maps.append({
            "x": xf[NIMG * c: NIMG * (c + 1)],
            "bias": np.ascontiguousarray(bias_full[NIMG * c: NIMG * (c + 1)]),
            "cm": cm,
            "cd": cd,
        })
    res = run_bass_kernel_spmd(nc, in_maps, core_ids=list(range(NCORES)),
                               trace=trace)
    out = np.concatenate([res.results[c]["out"][None] for c in range(NCORES)], 0)
    out = out.reshape(4, 256, OUT, OUT)
    return out, res


def kernel(x, bias, up_filter, down_filter):
    out, _ = _run(x, bias, up_filter, down_filter, trace=False)
    return out


def kernel_traced(x, bias, up_filter, down_filter):
    return _run(x, bias, up_filter, down_filter, trace=True)


# revision 10
# speedup vs baseline: 1.2011x; 1.2011x over previous
"""AliasFreeActivation Trainium2 kernel (v3: fp16 matmuls, banded down-path).

out = crop10(down2(leaky_relu(up4(x + bias)) * sqrt2))   [4,256,236,236]

Decomposition per (batch,channel) image (1024 images, 128 per core):
  leaky_relu(t)*s = 0.6*s*t + 0.4*s*|t|   (slope 0.2)
so with y = up4(xb):
  out = Down(0.4*sqrt2*|y|)  +  Down(0.6*sqrt2*y)
The second (linear) term collapses through the composed matrices
Mv = A@D so it never touches the big upsampled grid.

Stages (matmul contraction is always the SBUF partition dim; the image
data is the stationary lhsT so the kept axis lands on the output
partitions, chaining without transposes):
  s1  v1[w,ho]   = sum_h xb[h,w] A[h,ho]            1 MM  N=512
  sA  u1[w,hd]   = sum_h xb[h,w] Mv[h,hd]           1 MM  N=256   (linear)
  s2  p2[ho,wo]  = sum_w v1[w,ho] A2[w,wo]          4 MM  N=512   (A2=0.4*sqrt2*A)
  abs Y = |p2|                                      (one ACT/DVE pass)
  s3  z[wo,hd]   = sum_ho Y[ho,wo] D[ho,hd]        16 MM  banded N<=70
  s4  o[hd,wd]   = sum_wo z[wo,hd] D[wo,wd]         8 MM  banded N<=70
  sB  o += sum_w u1[w,hd] Mh[w,wd]                  2 MM  N=236   (Mh=0.6*sqrt2*Mv)
All matmul operands are fp16 (1 cycle/row at any N, FWL weight loads);
PSUM accumulation is fp32.
"""
import numpy as np

UP, DOWN, MARGIN, NEG_SLOPE = 4, 2, 10, 0.2
SQRT2 = 1.4142135623730951
H = W = 128
OUT = 236
NCORES = 8
NIMG = 128

# down-matrix window per 128-row K-chunk: D[s,o] nonzero for s in [2o-5,2o+6]
DWIN = [(0, 67), (61, 131), (125, 195), (189, 256)]
# s3 DoubleRow pair windows (ho chunks (2j,2j+1)), cropped to [10,246)
DWIN3 = [(10, 131), (125, 246)]

_cache = {}


def _build_nc(nimg=NIMG):
    import concourse.bacc as bacc
    import concourse.bass as bass
    import concourse.tile as tile
    from concourse import mybir

    F32 = mybir.dt.float32
    F16 = mybir.dt.float16
    F8 = mybir.dt.float8e4
    AF = mybir.ActivationFunctionType
    ALU = mybir.AluOpType
    DR = mybir.MatmulPerfMode.DoubleRow

    nc = bacc.Bacc("TRN2", target_bir_lowering=False)
    x_d = nc.dram_tensor("x", [nimg, H, W], F32, kind="ExternalInput")
    b_d = nc.dram_tensor("bias", [nimg], F32, kind="ExternalInput")
    c_d = nc.dram_tensor("cm", [128, 2048], F16, kind="ExternalInput")
    c8_d = nc.dram_tensor("cd", [128, 484], F8, kind="ExternalInput")
    o_d = nc.dram_tensor("out", [nimg, OUT, OUT], F32, kind="ExternalOutput")

    with tile.TileContext(nc) as tc:
        with (
            tc.tile_pool(name="const", bufs=1) as const,
            tc.tile_pool(name="xin", bufs=4) as xin,
            tc.tile_pool(name="xbp", bufs=2) as xbp,
            tc.tile_pool(name="v1p", bufs=2) as v1p,
            tc.tile_pool(name="u1p", bufs=2) as u1p,
            tc.tile_pool(name="yp", bufs=2) as yp,
            tc.tile_pool(name="zp", bufs=2) as zp,
            tc.tile_pool(name="op", bufs=4) as op_,
            tc.tile_pool(name="ps", bufs=2, space="PSUM") as ps,
            tc.tile_pool(name="ps3", bufs=3, space="PSUM") as ps3,
            tc.tile_pool(name="ps4", bufs=1, space="PSUM") as ps4,
        ):
            cm = const.tile([128, 2048], F16)
            nc.sync.dma_start(out=cm, in_=c_d[:])
            cd = const.tile([128, 4, 121], F8)
            nc.sync.dma_start(out=cd, in_=c8_d[:])
            A_sb = cm[:, 0:512]
            A2_sb = cm[:, 512:1024]                    # 0.4*sqrt2*A
            dw0 = 1024

            def D_sb(k):
                o0, o1 = DWIN[k]
                return cm[:, dw0 + 70 * k: dw0 + 70 * k + (o1 - o0)]

            mv0 = dw0 + 280                            # Mv [128,256]
            Mv_sb = cm[:, mv0: mv0 + 256]
            mh0 = mv0 + 256                            # Mh [128,236] (cropped)
            Mh_sb = cm[:, mh0: mh0 + 236]

            bb = const.tile([128, nimg], F32)
            nc.gpsimd.dma_start(
                out=bb,
                in_=bass.AP(tensor=b_d[:].tensor, offset=0,
                            ap=[[0, 128], [1, nimg]]),
            )

            # warm PE's clock on the const DMA lane
            pwarm = ps3.tile([128, 256], F32, name="p3")
            nc.tensor.matmul(out=pwarm[:32, :256], lhsT=cm[:, :32],
                             rhs=cm[:, :256], start=True, stop=True)

            for i in range(nimg):
                X = xin.tile([128, W], F32)
                nc.sync.dma_start(out=X, in_=x_d[i])
                Xb = xbp.tile([128, W], F16)
                nc.scalar.activation(out=Xb, in_=X, func=AF.Identity,
                                     bias=bb[:, i:i + 1], scale=1.0)

                # s1: up vertical
                P1 = ps.tile([128, 512], F32, name="p1")
                nc.tensor.matmul(out=P1, lhsT=Xb, rhs=A_sb,
                                 start=True, stop=True)
                V1 = v1p.tile([128, 512], F16)
                nc.vector.tensor_copy(out=V1, in_=P1)

                # sA: linear path, vertical compose
                PA = ps3.tile([128, 256], F32, name="p3")
                nc.tensor.matmul(out=PA, lhsT=Xb, rhs=Mv_sb,
                                 start=True, stop=True)
                U1 = u1p.tile([128, OUT], F16)
                nc.vector.tensor_copy(out=U1, in_=PA[:, MARGIN:MARGIN + OUT])

                # s2 + |.|: up horizontal then one-pass abs evacuation
                Y = yp.tile([128, 4, 512], F8)
                for m in range(4):
                    P2 = ps.tile([128, 512], F32, name="p2")
                    nc.tensor.matmul(out=P2, lhsT=V1[:, 128 * m:128 * (m + 1)],
                                     rhs=A2_sb, start=True, stop=True)
                    nc.scalar.activation(out=Y[:, m, :], in_=P2,
                                         func=AF.Abs, bias=0.0, scale=1.0)

                # s3: down vertical, fp8 DoubleRow over ho-chunk pairs
                Z = zp.tile([128, 4, OUT], F16)
                for m in range(4):
                    P3 = ps3.tile([128, 256], F32, name="p3")
                    for j in range(2):
                        o0, o1 = DWIN3[j]
                        nc.tensor.matmul(
                            out=P3[:, o0:o1],
                            lhsT=Y[:, 2 * j:2 * j + 2, 128 * m:128 * (m + 1)],
                            rhs=cd[:, 2 * j:2 * j + 2, :o1 - o0],
                            start=(j == 0), stop=(j == 1),
                            perf_mode=DR, skip_group_check=True)
                    nc.vector.tensor_copy(out=Z[:, m, :],
                                          in_=P3[:, MARGIN:MARGIN + OUT])

                # s4 + sB: down horizontal (banded) + linear-path accumulate
                for mo, (h0, h1) in enumerate(((0, 128), (128, OUT))):
                    rows = h1 - h0
                    P4 = ps4.tile([128, 256], F32, name="p4")
                    for k in range(4):
                        o0, o1 = DWIN[k]
                        nc.tensor.matmul(
                            out=P4[:rows, o0:o1],
                            lhsT=Z[:, k, h0:h1],
                            rhs=D_sb(k), start=(k == 0), stop=False)
                    # linear path accumulates into the same PSUM group
                    nc.tensor.matmul(
                        out=P4[:rows, MARGIN:MARGIN + OUT],
                        lhsT=U1[:, h0:h1],
                        rhs=Mh_sb, start=False, stop=True)
                    O = op_.tile([128, OUT], F32)
                    nc.vector.tensor_copy(out=O[:rows, :],
                                          in_=P4[:rows, MARGIN:MARGIN + OUT])
                    nc.sync.dma_start(out=o_d[i, h0:h1, :], in_=O[:rows, :])

    nc.finalize()
    return nc


def _filter_matrices(up_filter, down_filter):
    fu = np.asarray(up_filter, dtype=np.float64)
    fd = np.asarray(down_filter, dtype=np.float64)
    i = np.arange(128)[:, None]
    o = np.arange(512)[None, :]
    t = 10 + o - 4 * i
    A = np.where((t >= 0) & (t < 24), fu[np.clip(t, 0, 23)], 0.0)
    s = np.arange(512)[:, None]
    o2 = np.arange(256)[None, :]
    t2 = 6 + 2 * o2 - s
    D = np.where((t2 >= 0) & (t2 < 12), fd[np.clip(t2, 0, 11)], 0.0)
    return A, D


def _pack_consts(up_filter, down_filter):
    import ml_dtypes
    A, D = _filter_matrices(up_filter, down_filter)
    cm = np.zeros((128, 2048), dtype=np.float16)
    cm[:, 0:512] = A.astype(np.float16)
    cm[:, 512:1024] = (A * (0.4 * SQRT2)).astype(np.float16)
    dw0 = 1024
    for k, (o0, o1) in enumerate(DWIN):
        cm[:, dw0 + 70 * k: dw0 + 70 * k + (o1 - o0)] = \
            D[128 * k:128 * (k + 1), o0:o1].astype(np.float16)
    Mv = A @ D
    mv0 = dw0 + 280
    cm[:, mv0: mv0 + 256] = Mv.astype(np.float16)
    mh0 = mv0 + 256
    cm[:, mh0: mh0 + 236] = (Mv * (0.6 * SQRT2))[:, 10:246].astype(np.float16)
    cd = np.zeros((128, 4, 121), dtype=ml_dtypes.float8_e4m3)
    for j, (o0, o1) in enumerate(DWIN3):
        for sub in range(2):
            k = 2 * j + sub
            cd[:, 2 * j + sub, :o1 - o0] = \
                D[128 * k:128 * (k + 1), o0:o1].astype(ml_dtypes.float8_e4m3)
    return cm, cd.reshape(128, 484)


def _run(x, bias, up_filter, down_filter, trace=False):
    from concourse.bass_utils import run_bass_kernel_spmd

    if "nc" not in _cache:
        _cache["nc"] = _build_nc()
    nc = _cache["nc"]

    cm, cd = _pack_consts(up_filter, down_filter)
    xf = np.ascontiguousarray(np.asarray(x, dtype=np.float32)
                              .reshape(NCORES * NIMG, H, W))
    bias = np.asarray(bias, dtype=np.float32)
    bias_full = np.tile(bias, (NCORES * NIMG) // bias.shape[0])

    in_maps = []
    for c in range(NCORES):
        in_maps.append({
            "x": xf[NIMG * c: NIMG * (c + 1)],
            "bias": np.ascontiguousarray(bias_full[NIMG * c: NIMG * (c + 1)]),
            "cm": cm,
            "cd": cd,
        })
    res = run_bass_kernel_spmd(nc, in_maps, core_ids=list(range(NCORES)),
                               trace=trace)
    out = np.concatenate([res.results[c]["out"][None] for c in range(NCORES)], 0)
    out = out.reshape(4, 256, OUT, OUT)
    return out, res


def kernel(x, bias, up_filter, down_filter):
    out, _ = _run(x, bias, up_filter, down_filter, trace=False)
    return out


def kernel_traced(x, bias, up_filter, down_filter):
    return _run(x, bias, up_filter, down_filter, trace=True)



# revision 11
# speedup vs baseline: 1.2189x; 1.0148x over previous
"""AliasFreeActivation Trainium2 kernel (v3: fp16 matmuls, banded down-path).

out = crop10(down2(leaky_relu(up4(x + bias)) * sqrt2))   [4,256,236,236]

Decomposition per (batch,channel) image (1024 images, 128 per core):
  leaky_relu(t)*s = 0.6*s*t + 0.4*s*|t|   (slope 0.2)
so with y = up4(xb):
  out = Down(0.4*sqrt2*|y|)  +  Down(0.6*sqrt2*y)
The second (linear) term collapses through the composed matrices
Mv = A@D so it never touches the big upsampled grid.

Stages (matmul contraction is always the SBUF partition dim; the image
data is the stationary lhsT so the kept axis lands on the output
partitions, chaining without transposes):
  s1  v1[w,ho]   = sum_h xb[h,w] A[h,ho]            1 MM  N=512
  sA  u1[w,hd]   = sum_h xb[h,w] Mv[h,hd]           1 MM  N=256   (linear)
  s2  p2[ho,wo]  = sum_w v1[w,ho] A2[w,wo]          4 MM  N=512   (A2=0.4*sqrt2*A)
  abs Y = |p2|                                      (one ACT/DVE pass)
  s3  z[wo,hd]   = sum_ho Y[ho,wo] D[ho,hd]        16 MM  banded N<=70
  s4  o[hd,wd]   = sum_wo z[wo,hd] D[wo,wd]         8 MM  banded N<=70
  sB  o += sum_w u1[w,hd] Mh[w,wd]                  2 MM  N=236   (Mh=0.6*sqrt2*Mv)
All matmul operands are fp16 (1 cycle/row at any N, FWL weight loads);
PSUM accumulation is fp32.
"""
import numpy as np

UP, DOWN, MARGIN, NEG_SLOPE = 4, 2, 10, 0.2
SQRT2 = 1.4142135623730951
H = W = 128
OUT = 236
NCORES = 8
NIMG = 128

# down-matrix window per 128-row K-chunk: D[s,o] nonzero for s in [2o-5,2o+6]
DWIN = [(0, 67), (61, 131), (125, 195), (189, 256)]

_cache = {}


def _build_nc(nimg=NIMG):
    import concourse.bacc as bacc
    import concourse.bass as bass
    import concourse.tile as tile
    from concourse import mybir

    F32 = mybir.dt.float32
    F16 = mybir.dt.float16
    AF = mybir.ActivationFunctionType
    ALU = mybir.AluOpType

    nc = bacc.Bacc("TRN2", target_bir_lowering=False)
    x_d = nc.dram_tensor("x", [nimg, H, W], F32, kind="ExternalInput")
    b_d = nc.dram_tensor("bias", [nimg], F32, kind="ExternalInput")
    c_d = nc.dram_tensor("cm", [128, 2048], F16, kind="ExternalInput")
    o_d = nc.dram_tensor("out", [nimg, OUT, OUT], F32, kind="ExternalOutput")

    with tile.TileContext(nc) as tc:
        with (
            tc.tile_pool(name="const", bufs=1) as const,
            tc.tile_pool(name="xin", bufs=4) as xin,
            tc.tile_pool(name="xbp", bufs=2) as xbp,
            tc.tile_pool(name="v1p", bufs=2) as v1p,
            tc.tile_pool(name="u1p", bufs=2) as u1p,
            tc.tile_pool(name="yp", bufs=2) as yp,
            tc.tile_pool(name="zp", bufs=2) as zp,
            tc.tile_pool(name="op", bufs=4) as op_,
            tc.tile_pool(name="ps", bufs=2, space="PSUM") as ps,
            tc.tile_pool(name="ps3", bufs=3, space="PSUM") as ps3,
            tc.tile_pool(name="ps4", bufs=1, space="PSUM") as ps4,
        ):
            cm = const.tile([128, 2048], F16)
            nc.sync.dma_start(out=cm, in_=c_d[:])
            A_sb = cm[:, 0:512]
            A2_sb = cm[:, 512:1024]                    # 0.4*sqrt2*A
            dw0 = 1024

            def D_sb(k):
                o0, o1 = DWIN[k]
                return cm[:, dw0 + 70 * k: dw0 + 70 * k + (o1 - o0)]

            mv0 = dw0 + 280                            # Mv [128,256]
            Mv_sb = cm[:, mv0: mv0 + 256]
            mh0 = mv0 + 256                            # Mh [128,236] (cropped)
            Mh_sb = cm[:, mh0: mh0 + 236]

            bb = const.tile([128, nimg], F32)
            nc.gpsimd.dma_start(
                out=bb,
                in_=bass.AP(tensor=b_d[:].tensor, offset=0,
                            ap=[[0, 128], [1, nimg]]),
            )

            # warm PE's clock on the const DMA lane
            pwarm = ps3.tile([128, 256], F32, name="p3")
            nc.tensor.matmul(out=pwarm[:32, :256], lhsT=cm[:, :32],
                             rhs=cm[:, :256], start=True, stop=True)

            for i in range(nimg):
                X = xin.tile([128, W], F32)
                nc.sync.dma_start(out=X, in_=x_d[i])
                Xb = xbp.tile([128, W], F16)
                nc.scalar.activation(out=Xb, in_=X, func=AF.Identity,
                                     bias=bb[:, i:i + 1], scale=1.0)

                # s1: up vertical
                P1 = ps.tile([128, 512], F32, name="p1")
                nc.tensor.matmul(out=P1, lhsT=Xb, rhs=A_sb,
                                 start=True, stop=True)
                V1 = v1p.tile([128, 512], F16)
                nc.vector.tensor_copy(out=V1[:, 0:128], in_=P1[:, 0:128])
                nc.vector.tensor_copy(out=V1[:, 128:512], in_=P1[:, 128:512])

                # sA: linear path, vertical compose
                PA = ps3.tile([128, 256], F32, name="p3")
                nc.tensor.matmul(out=PA, lhsT=Xb, rhs=Mv_sb,
                                 start=True, stop=True)
                U1 = u1p.tile([128, OUT], F16)
                nc.vector.tensor_copy(out=U1, in_=PA[:, MARGIN:MARGIN + OUT])

                # s2 + |.|: up horizontal then one-pass abs evacuation
                Y = yp.tile([128, 4, 512], F16)
                for m in range(4):
                    P2 = ps.tile([128, 512], F32, name="p2")
                    nc.tensor.matmul(out=P2, lhsT=V1[:, 128 * m:128 * (m + 1)],
                                     rhs=A2_sb, start=True, stop=True)
                    nc.scalar.activation(out=Y[:, m, :], in_=P2,
                                         func=AF.Abs, bias=0.0, scale=1.0)

                # s3: down vertical (banded)
                Z = zp.tile([128, 4, OUT], F16)
                for m in range(4):
                    P3 = ps3.tile([128, 256], F32, name="p3")
                    for k in range(4):
                        o0, o1 = DWIN[k]
                        nc.tensor.matmul(
                            out=P3[:, o0:o1],
                            lhsT=Y[:, k, 128 * m:128 * (m + 1)],
                            rhs=D_sb(k), start=(k == 0), stop=(k == 3))
                    nc.vector.tensor_copy(out=Z[:, m, :],
                                          in_=P3[:, MARGIN:MARGIN + OUT])

                # s4 + sB: down horizontal (banded) + linear-path accumulate
                # both row-blocks in one PSUM bank (slot pair, single start)
                P4 = ps4.tile([128, 2, 256], F32, name="p4")
                for mo, (h0, h1) in enumerate(((0, 128), (128, OUT))):
                    rows = h1 - h0
                    for k in range(4):
                        o0, o1 = DWIN[k]
                        nc.tensor.matmul(
                            out=P4[:rows, mo, o0:o1],
                            lhsT=Z[:, k, h0:h1],
                            rhs=D_sb(k), start=(mo == 0 and k == 0),
                            stop=False, skip_group_check=True)
                    # linear path accumulates into the same PSUM group
                    nc.tensor.matmul(
                        out=P4[:rows, mo, MARGIN:MARGIN + OUT],
                        lhsT=U1[:, h0:h1],
                        rhs=Mh_sb, start=False, stop=(mo == 1),
                        skip_group_check=True)
                O = op_.tile([128, 2, OUT], F32)
                nc.vector.tensor_copy(out=O, in_=P4[:, :, MARGIN:MARGIN + OUT])
                nc.sync.dma_start(out=o_d[i, 0:128, :], in_=O[:, 0, :])
                nc.sync.dma_start(out=o_d[i, 128:OUT, :], in_=O[:OUT - 128, 1, :])

    nc.finalize()
    return nc


def _filter_matrices(up_filter, down_filter):
    fu = np.asarray(up_filter, dtype=np.float64)
    fd = np.asarray(down_filter, dtype=np.float64)
    i = np.arange(128)[:, None]
    o = np.arange(512)[None, :]
    t = 10 + o - 4 * i
    A = np.where((t >= 0) & (t < 24), fu[np.clip(t, 0, 23)], 0.0)
    s = np.arange(512)[:, None]
    o2 = np.arange(256)[None, :]
    t2 = 6 + 2 * o2 - s
    D = np.where((t2 >= 0) & (t2 < 12), fd[np.clip(t2, 0, 11)], 0.0)
    return A, D


def _pack_consts(up_filter, down_filter):
    A, D = _filter_matrices(up_filter, down_filter)
    cm = np.zeros((128, 2048), dtype=np.float16)
    cm[:, 0:512] = A.astype(np.float16)
    cm[:, 512:1024] = (A * (0.4 * SQRT2)).astype(np.float16)
    dw0 = 1024
    for k, (o0, o1) in enumerate(DWIN):
        cm[:, dw0 + 70 * k: dw0 + 70 * k + (o1 - o0)] = \
            D[128 * k:128 * (k + 1), o0:o1].astype(np.float16)
    Mv = A @ D
    mv0 = dw0 + 280
    cm[:, mv0: mv0 + 256] = Mv.astype(np.float16)
    mh0 = mv0 + 256
    cm[:, mh0: mh0 + 236] = (Mv * (0.6 * SQRT2))[:, 10:246].astype(np.float16)
    return cm


def _run(x, bias, up_filter, down_filter, trace=False):
    from concourse.bass_utils import run_bass_kernel_spmd

    if "nc" not in _cache:
        _cache["nc"] = _build_nc()
    nc = _cache["nc"]

    cm = _pack_consts(up_filter, down_filter)
    xf = np.ascontiguousarray(np.asarray(x, dtype=np.float32)
                              .reshape(NCORES * NIMG, H, W))
    bias = np.asarray(bias, dtype=np.float32)
    bias_full = np.tile(bias, (NCORES * NIMG) // bias.shape[0])

    in_maps = []
    for c in range(NCORES):
        in_maps.append({
            "x": xf[NIMG * c: NIMG * (c + 1)],
            "bias": np.ascontiguousarray(bias_full[NIMG * c: NIMG * (c + 1)]),
            "cm": cm,
        })
    res = run_bass_kernel_spmd(nc, in_maps, core_ids=list(range(NCORES)),
                               trace=trace)
    out = np.concatenate([res.results[c]["out"][None] for c in range(NCORES)], 0)
    out = out.reshape(4, 256, OUT, OUT)
    return out, res


def kernel(x, bias, up_filter, down_filter):
    out, _ = _run(x, bias, up_filter, down_filter, trace=False)
    return out


def kernel_traced(x, bias, up_filter, down_filter):
    return _run(x, bias, up_filter, down_filter, trace=True)



# revision 12
# speedup vs baseline: 1.3420x; 1.1010x over previous
"""AliasFreeActivation Trainium2 kernel (v3: fp16 matmuls, banded down-path).

out = crop10(down2(leaky_relu(up4(x + bias)) * sqrt2))   [4,256,236,236]

Decomposition per (batch,channel) image (1024 images, 128 per core):
  leaky_relu(t)*s = 0.6*s*t + 0.4*s*|t|   (slope 0.2)
so with y = up4(xb):
  out = Down(0.4*sqrt2*|y|)  +  Down(0.6*sqrt2*y)
The second (linear) term collapses through the composed matrices
Mv = A@D so it never touches the big upsampled grid.

Stages (matmul contraction is always the SBUF partition dim; the image
data is the stationary lhsT so the kept axis lands on the output
partitions, chaining without transposes):
  s1  v1[w,ho]   = sum_h xb[h,w] A[h,ho]            1 MM  N=512
  sA  u1[w,hd]   = sum_h xb[h,w] Mv[h,hd]           1 MM  N=256   (linear)
  s2  p2[ho,wo]  = sum_w v1[w,ho] A2[w,wo]          4 MM  N=512   (A2=0.4*sqrt2*A)
  abs Y = |p2|                                      (one ACT/DVE pass)
  s3  z[wo,hd]   = sum_ho Y[ho,wo] D[ho,hd]        16 MM  banded N<=70
  s4  o[hd,wd]   = sum_wo z[wo,hd] D[wo,wd]         8 MM  banded N<=70
  sB  o += sum_w u1[w,hd] Mh[w,wd]                  2 MM  N=236   (Mh=0.6*sqrt2*Mv)
All matmul operands are fp16 (1 cycle/row at any N, FWL weight loads);
PSUM accumulation is fp32.
"""
import numpy as np

UP, DOWN, MARGIN, NEG_SLOPE = 4, 2, 10, 0.2
SQRT2 = 1.4142135623730951
H = W = 128
OUT = 236
NCORES = 8
NIMG = 128

# down-matrix window per 128-row K-chunk: D[s,o] nonzero for s in [2o-5,2o+6]
DWIN = [(0, 67), (61, 131), (125, 195), (189, 256)]

_cache = {}


def _build_nc(nimg=NIMG):
    import concourse.bacc as bacc
    import concourse.bass as bass
    import concourse.tile as tile
    from concourse import mybir

    F32 = mybir.dt.float32
    F16 = mybir.dt.float16
    AF = mybir.ActivationFunctionType
    ALU = mybir.AluOpType

    nc = bacc.Bacc("TRN2", target_bir_lowering=False)
    x_d = nc.dram_tensor("x", [nimg, H, W], F32, kind="ExternalInput")
    b_d = nc.dram_tensor("bias", [nimg], F32, kind="ExternalInput")
    c_d = nc.dram_tensor("cm", [128, 2048], F16, kind="ExternalInput")
    o_d = nc.dram_tensor("out", [nimg, OUT, OUT], F32, kind="ExternalOutput")

    with tile.TileContext(nc) as tc:
        with (
            tc.tile_pool(name="const", bufs=1) as const,
            tc.tile_pool(name="xin", bufs=4) as xin,
            tc.tile_pool(name="xbp", bufs=2) as xbp,
            tc.tile_pool(name="v1p", bufs=2) as v1p,
            tc.tile_pool(name="u1p", bufs=2) as u1p,
            tc.tile_pool(name="yp", bufs=2) as yp,
            tc.tile_pool(name="zp", bufs=2) as zp,
            tc.tile_pool(name="op", bufs=4) as op_,
            tc.tile_pool(name="ps", bufs=2, space="PSUM") as ps,
            tc.tile_pool(name="ps3", bufs=3, space="PSUM") as ps3,
            tc.tile_pool(name="ps4", bufs=1, space="PSUM") as ps4,
        ):
            cm = const.tile([128, 2048], F16)
            nc.sync.dma_start(out=cm, in_=c_d[:])
            A_sb = cm[:, 0:512]
            A2_sb = cm[:, 512:1024]                    # 0.4*sqrt2*A
            dw0 = 1024

            def D_sb(k):
                o0, o1 = DWIN[k]
                return cm[:, dw0 + 70 * k: dw0 + 70 * k + (o1 - o0)]

            mv0 = dw0 + 280                            # Mv [128,256]
            Mv_sb = cm[:, mv0: mv0 + 256]
            mh0 = mv0 + 256                            # Mh [128,236] (cropped)
            Mh_sb = cm[:, mh0: mh0 + 236]

            bb = const.tile([128, nimg], F32)
            nc.gpsimd.dma_start(
                out=bb,
                in_=bass.AP(tensor=b_d[:].tensor, offset=0,
                            ap=[[0, 128], [1, nimg]]),
            )

            # warm PE's clock on the const DMA lane
            pwarm = ps3.tile([128, 256], F32, name="p3")
            nc.tensor.matmul(out=pwarm[:32, :256], lhsT=cm[:, :32],
                             rhs=cm[:, :256], start=True, stop=True)

            for i in range(nimg):
                X = xin.tile([128, W], F32)
                nc.sync.dma_start(out=X, in_=x_d[i])
                Xb = xbp.tile([128, W], F16)
                nc.scalar.activation(out=Xb, in_=X, func=AF.Identity,
                                     bias=bb[:, i:i + 1], scale=1.0)

                # s1: up vertical
                P1 = ps.tile([128, 512], F32, name="p1")
                nc.tensor.matmul(out=P1, lhsT=Xb, rhs=A_sb,
                                 start=True, stop=True)
                V1 = v1p.tile([128, 512], F16)
                nc.vector.tensor_copy(out=V1, in_=P1)

                # sA: linear path, vertical compose
                PA = ps3.tile([128, 256], F32, name="p3")
                nc.tensor.matmul(out=PA, lhsT=Xb, rhs=Mv_sb,
                                 start=True, stop=True)
                U1 = u1p.tile([128, OUT], F16)
                nc.vector.tensor_copy(out=U1, in_=PA[:, MARGIN:MARGIN + OUT])

                # s2 + |.|: up horizontal then one-pass abs evacuation
                Y = yp.tile([128, 4, 512], F16)
                for m in range(4):
                    P2 = ps.tile([128, 512], F32, name="p2")
                    nc.tensor.matmul(out=P2, lhsT=V1[:, 128 * m:128 * (m + 1)],
                                     rhs=A2_sb, start=True, stop=True)
                    nc.scalar.activation(out=Y[:, m, :], in_=P2,
                                         func=AF.Abs, bias=0.0, scale=1.0)

                # s3: down vertical (banded)
                Z = zp.tile([128, 4, OUT], F16)
                for m in range(4):
                    P3 = ps3.tile([128, 256], F32, name="p3")
                    for k in range(4):
                        o0, o1 = DWIN[k]
                        nc.tensor.matmul(
                            out=P3[:, o0:o1],
                            lhsT=Y[:, k, 128 * m:128 * (m + 1)],
                            rhs=D_sb(k), start=(k == 0), stop=(k == 3))
                    nc.vector.tensor_copy(out=Z[:, m, :],
                                          in_=P3[:, MARGIN:MARGIN + OUT])

                # s4 + sB: down horizontal (banded) + linear-path accumulate
                # both row-blocks in one PSUM bank (slot pair, single start)
                P4 = ps4.tile([128, 2, 256], F32, name="p4")
                for mo, (h0, h1) in enumerate(((0, 128), (128, OUT))):
                    rows = h1 - h0
                    for k in range(4):
                        o0, o1 = DWIN[k]
                        nc.tensor.matmul(
                            out=P4[:rows, mo, o0:o1],
                            lhsT=Z[:, k, h0:h1],
                            rhs=D_sb(k), start=(mo == 0 and k == 0),
                            stop=False, skip_group_check=True)
                    # linear path accumulates into the same PSUM group
                    nc.tensor.matmul(
                        out=P4[:rows, mo, MARGIN:MARGIN + OUT],
                        lhsT=U1[:, h0:h1],
                        rhs=Mh_sb, start=False, stop=(mo == 1),
                        skip_group_check=True)
                O = op_.tile([128, 2, OUT], F32)
                nc.vector.tensor_copy(out=O, in_=P4[:, :, MARGIN:MARGIN + OUT])
                nc.sync.dma_start(out=o_d[i, 0:128, :], in_=O[:, 0, :])
                nc.sync.dma_start(out=o_d[i, 128:OUT, :], in_=O[:OUT - 128, 1, :])

    nc.finalize()
    return nc


def _filter_matrices(up_filter, down_filter):
    fu = np.asarray(up_filter, dtype=np.float64)
    fd = np.asarray(down_filter, dtype=np.float64)
    i = np.arange(128)[:, None]
    o = np.arange(512)[None, :]
    t = 10 + o - 4 * i
    A = np.where((t >= 0) & (t < 24), fu[np.clip(t, 0, 23)], 0.0)
    s = np.arange(512)[:, None]
    o2 = np.arange(256)[None, :]
    t2 = 6 + 2 * o2 - s
    D = np.where((t2 >= 0) & (t2 < 12), fd[np.clip(t2, 0, 11)], 0.0)
    return A, D


def _pack_consts(up_filter, down_filter):
    A, D = _filter_matrices(up_filter, down_filter)
    cm = np.zeros((128, 2048), dtype=np.float16)
    cm[:, 0:512] = A.astype(np.float16)
    cm[:, 512:1024] = (A * (0.4 * SQRT2)).astype(np.float16)
    dw0 = 1024
    for k, (o0, o1) in enumerate(DWIN):
        cm[:, dw0 + 70 * k: dw0 + 70 * k + (o1 - o0)] = \
            D[128 * k:128 * (k + 1), o0:o1].astype(np.float16)
    Mv = A @ D
    mv0 = dw0 + 280
    cm[:, mv0: mv0 + 256] = Mv.astype(np.float16)
    mh0 = mv0 + 256
    cm[:, mh0: mh0 + 236] = (Mv * (0.6 * SQRT2))[:, 10:246].astype(np.float16)
    return cm


def _run(x, bias, up_filter, down_filter, trace=False):
    from concourse.bass_utils import run_bass_kernel_spmd

    if "nc" not in _cache:
        _cache["nc"] = _build_nc()
    nc = _cache["nc"]

    cm = _pack_consts(up_filter, down_filter)
    xf = np.ascontiguousarray(np.asarray(x, dtype=np.float32)
                              .reshape(NCORES * NIMG, H, W))
    bias = np.asarray(bias, dtype=np.float32)
    bias_full = np.tile(bias, (NCORES * NIMG) // bias.shape[0])

    in_maps = []
    for c in range(NCORES):
        in_maps.append({
            "x": xf[NIMG * c: NIMG * (c + 1)],
            "bias": np.ascontiguousarray(bias_full[NIMG * c: NIMG * (c + 1)]),
            "cm": cm,
        })
    res = run_bass_kernel_spmd(nc, in_maps, core_ids=list(range(NCORES)),
                               trace=trace)
    out = np.concatenate([res.results[c]["out"][None] for c in range(NCORES)], 0)
    out = out.reshape(4, 256, OUT, OUT)
    return out, res


def kernel(x, bias, up_filter, down_filter):
    out, _ = _run(x, bias, up_filter, down_filter, trace=False)
    return out


def kernel_traced(x, bias, up_filter, down_filter):
    return _run(x, bias, up_filter, down_filter, trace=True)



# revision 13
# speedup vs baseline: 1.4410x; 1.0738x over previous
"""AliasFreeActivation Trainium2 kernel (v3: fp16 matmuls, banded down-path).

out = crop10(down2(leaky_relu(up4(x + bias)) * sqrt2))   [4,256,236,236]

Decomposition per (batch,channel) image (1024 images, 128 per core):
  leaky_relu(t)*s = 0.6*s*t + 0.4*s*|t|   (slope 0.2)
so with y = up4(xb):
  out = Down(0.4*sqrt2*|y|)  +  Down(0.6*sqrt2*y)
The second (linear) term collapses through the composed matrices
Mv = A@D so it never touches the big upsampled grid.

Stages (matmul contraction is always the SBUF partition dim; the image
data is the stationary lhsT so the kept axis lands on the output
partitions, chaining without transposes):
  s1  v1[w,ho]   = sum_h xb[h,w] A[h,ho]            1 MM  N=512
  sA  u1[w,hd]   = sum_h xb[h,w] Mv[h,hd]           1 MM  N=256   (linear)
  s2  p2[ho,wo]  = sum_w v1[w,ho] A2[w,wo]          4 MM  N=512   (A2=0.4*sqrt2*A)
  abs Y = |p2|                                      (one ACT/DVE pass)
  s3  z[wo,hd]   = sum_ho Y[ho,wo] D[ho,hd]        16 MM  banded N<=70
  s4  o[hd,wd]   = sum_wo z[wo,hd] D[wo,wd]         8 MM  banded N<=70
  sB  o += sum_w u1[w,hd] Mh[w,wd]                  2 MM  N=236   (Mh=0.6*sqrt2*Mv)
All matmul operands are fp16 (1 cycle/row at any N, FWL weight loads);
PSUM accumulation is fp32.
"""
import numpy as np

UP, DOWN, MARGIN, NEG_SLOPE = 4, 2, 10, 0.2
SQRT2 = 1.4142135623730951
H = W = 128
OUT = 236
NCORES = 8
NIMG = 128

# down-matrix window per 128-row K-chunk: D[s,o] nonzero for s in [2o-5,2o+6]
DWIN = [(0, 67), (61, 131), (125, 195), (189, 256)]

_cache = {}


def _build_nc(nimg=NIMG):
    import concourse.bacc as bacc
    import concourse.bass as bass
    import concourse.tile as tile
    from concourse import mybir

    F32 = mybir.dt.float32
    F16 = mybir.dt.float16
    AF = mybir.ActivationFunctionType
    ALU = mybir.AluOpType

    nc = bacc.Bacc("TRN2", target_bir_lowering=False)
    x_d = nc.dram_tensor("x", [nimg, H, W], F32, kind="ExternalInput")
    b_d = nc.dram_tensor("bias", [nimg], F32, kind="ExternalInput")
    c_d = nc.dram_tensor("cm", [128, 2048], F16, kind="ExternalInput")
    o_d = nc.dram_tensor("out", [nimg, OUT, OUT], F32, kind="ExternalOutput")

    with tile.TileContext(nc) as tc:
        with (
            tc.tile_pool(name="const", bufs=1) as const,
            tc.tile_pool(name="xin", bufs=4) as xin,
            tc.tile_pool(name="xbp", bufs=2) as xbp,
            tc.tile_pool(name="v1p", bufs=2) as v1p,
            tc.tile_pool(name="u1p", bufs=2) as u1p,
            tc.tile_pool(name="yp", bufs=2) as yp,
            tc.tile_pool(name="zp", bufs=2) as zp,
            tc.tile_pool(name="op", bufs=4) as op_,
            tc.tile_pool(name="ps", bufs=2, space="PSUM") as ps,
            tc.tile_pool(name="ps3", bufs=3, space="PSUM") as ps3,
            tc.tile_pool(name="ps4", bufs=1, space="PSUM") as ps4,
        ):
            cm = const.tile([128, 2048], F16)
            nc.sync.dma_start(out=cm, in_=c_d[:])
            A_sb = cm[:, 0:512]
            A2_sb = cm[:, 512:1024]                    # 0.4*sqrt2*A
            dw0 = 1024

            def D_sb(k):
                o0, o1 = DWIN[k]
                return cm[:, dw0 + 70 * k: dw0 + 70 * k + (o1 - o0)]

            mv0 = dw0 + 280                            # Mv [128,256]
            Mv_sb = cm[:, mv0: mv0 + 256]
            mh0 = mv0 + 256                            # Mh [128,236] (cropped)
            Mh_sb = cm[:, mh0: mh0 + 236]

            bb = const.tile([128, nimg], F32)
            nc.gpsimd.dma_start(
                out=bb,
                in_=bass.AP(tensor=b_d[:].tensor, offset=0,
                            ap=[[0, 128], [1, nimg]]),
            )

            # warm PE's clock on the const DMA lane
            pwarm = ps3.tile([128, 256], F32, name="p3")
            nc.tensor.matmul(out=pwarm[:32, :256], lhsT=cm[:, :32],
                             rhs=cm[:, :256], start=True, stop=True)

            for i in range(nimg):
                X = xin.tile([128, W], F32)
                nc.sync.dma_start(out=X, in_=x_d[i])
                Xb = xbp.tile([128, W], F16)
                nc.scalar.activation(out=Xb, in_=X, func=AF.Identity,
                                     bias=bb[:, i:i + 1], scale=1.0)

                # s1: up vertical
                P1 = ps.tile([128, 512], F32, name="p1")
                nc.tensor.matmul(out=P1, lhsT=Xb, rhs=A_sb,
                                 start=True, stop=True)
                V1 = v1p.tile([128, 512], F16)
                nc.vector.tensor_copy(out=V1, in_=P1)

                # sA: linear path, vertical compose
                PA = ps3.tile([128, 256], F32, name="p3")
                nc.tensor.matmul(out=PA, lhsT=Xb, rhs=Mv_sb,
                                 start=True, stop=True)
                U1 = u1p.tile([128, OUT], F16)
                nc.vector.tensor_copy(out=U1, in_=PA[:, MARGIN:MARGIN + OUT])

                # s2 + |.|: up horizontal then one-pass abs evacuation
                Y = yp.tile([128, 4, 512], F16)
                for m in range(4):
                    P2 = ps.tile([128, 512], F32, name="p2")
                    nc.tensor.matmul(out=P2, lhsT=V1[:, 128 * m:128 * (m + 1)],
                                     rhs=A2_sb, start=True, stop=True)
                    nc.scalar.activation(out=Y[:, m, :], in_=P2,
                                         func=AF.Abs, bias=0.0, scale=1.0)

                # s3: down vertical (banded); wo-chunk pairs share a PSUM
                # bank so each pair drains with one batched cast
                Z = zp.tile([128, 4, OUT], F16)
                for half in range(2):
                    P3 = ps3.tile([128, 2, 256], F32, name="p3")
                    for s in range(2):
                        m = 2 * half + s
                        for k in range(4):
                            o0, o1 = DWIN[k]
                            nc.tensor.matmul(
                                out=P3[:, s, o0:o1],
                                lhsT=Y[:, k, 128 * m:128 * (m + 1)],
                                rhs=D_sb(k), start=(s == 0 and k == 0),
                                stop=(k == 3), skip_group_check=True)
                    nc.vector.tensor_copy(
                        out=Z[:, 2 * half:2 * half + 2, :],
                        in_=P3[:, :, MARGIN:MARGIN + OUT])

                # s4 + sB: down horizontal (banded) + linear-path accumulate
                # both row-blocks in one PSUM bank (slot pair, single start)
                P4 = ps4.tile([128, 2, 256], F32, name="p4")
                for mo, (h0, h1) in enumerate(((0, 128), (128, OUT))):
                    rows = h1 - h0
                    for k in range(4):
                        o0, o1 = DWIN[k]
                        nc.tensor.matmul(
                            out=P4[:rows, mo, o0:o1],
                            lhsT=Z[:, k, h0:h1],
                            rhs=D_sb(k), start=(mo == 0 and k == 0),
                            stop=False, skip_group_check=True)
                    # linear path accumulates into the same PSUM group
                    nc.tensor.matmul(
                        out=P4[:rows, mo, MARGIN:MARGIN + OUT],
                        lhsT=U1[:, h0:h1],
                        rhs=Mh_sb, start=False, stop=(mo == 1),
                        skip_group_check=True)
                O = op_.tile([128, 2, OUT], F32)
                nc.vector.tensor_copy(out=O, in_=P4[:, :, MARGIN:MARGIN + OUT])
                nc.sync.dma_start(out=o_d[i, 0:128, :], in_=O[:, 0, :])
                nc.sync.dma_start(out=o_d[i, 128:OUT, :], in_=O[:OUT - 128, 1, :])

    nc.finalize()
    return nc


def _filter_matrices(up_filter, down_filter):
    fu = np.asarray(up_filter, dtype=np.float64)
    fd = np.asarray(down_filter, dtype=np.float64)
    i = np.arange(128)[:, None]
    o = np.arange(512)[None, :]
    t = 10 + o - 4 * i
    A = np.where((t >= 0) & (t < 24), fu[np.clip(t, 0, 23)], 0.0)
    s = np.arange(512)[:, None]
    o2 = np.arange(256)[None, :]
    t2 = 6 + 2 * o2 - s
    D = np.where((t2 >= 0) & (t2 < 12), fd[np.clip(t2, 0, 11)], 0.0)
    return A, D


def _pack_consts(up_filter, down_filter):
    A, D = _filter_matrices(up_filter, down_filter)
    cm = np.zeros((128, 2048), dtype=np.float16)
    cm[:, 0:512] = A.astype(np.float16)
    cm[:, 512:1024] = (A * (0.4 * SQRT2)).astype(np.float16)
    dw0 = 1024
    for k, (o0, o1) in enumerate(DWIN):
        cm[:, dw0 + 70 * k: dw0 + 70 * k + (o1 - o0)] = \
            D[128 * k:128 * (k + 1), o0:o1].astype(np.float16)
    Mv = A @ D
    mv0 = dw0 + 280
    cm[:, mv0: mv0 + 256] = Mv.astype(np.float16)
    mh0 = mv0 + 256
    cm[:, mh0: mh0 + 236] = (Mv * (0.6 * SQRT2))[:, 10:246].astype(np.float16)
    return cm


def _run(x, bias, up_filter, down_filter, trace=False):
    from concourse.bass_utils import run_bass_kernel_spmd

    if "nc" not in _cache:
        _cache["nc"] = _build_nc()
    nc = _cache["nc"]

    cm = _pack_consts(up_filter, down_filter)
    xf = np.ascontiguousarray(np.asarray(x, dtype=np.float32)
                              .reshape(NCORES * NIMG, H, W))
    bias = np.asarray(bias, dtype=np.float32)
    bias_full = np.tile(bias, (NCORES * NIMG) // bias.shape[0])

    in_maps = []
    for c in range(NCORES):
        in_maps.append({
            "x": xf[NIMG * c: NIMG * (c + 1)],
            "bias": np.ascontiguousarray(bias_full[NIMG * c: NIMG * (c + 1)]),
            "cm": cm,
        })
    res = run_bass_kernel_spmd(nc, in_maps, core_ids=list(range(NCORES)),
                               trace=trace)
    out = np.concatenate([res.results[c]["out"][None] for c in range(NCORES)], 0)
    out = out.reshape(4, 256, OUT, OUT)
    return out, res


def kernel(x, bias, up_filter, down_filter):
    out, _ = _run(x, bias, up_filter, down_filter, trace=False)
    return out


def kernel_traced(x, bias, up_filter, down_filter):
    return _run(x, bias, up_filter, down_filter, trace=True)



# revision 14
# speedup vs baseline: 1.4454x; 1.0031x over previous
"""AliasFreeActivation Trainium2 kernel (v13: fp16 matmuls, banded down-path,
paired PSUM banks with batched evacuation on the down path).

out = crop10(down2(leaky_relu(up4(x + bias)) * sqrt2))   [4,256,236,236]

Decomposition per (batch,channel) image (1024 images, 128 per core):
  leaky_relu(t)*s = 0.6*s*t + 0.4*s*|t|   (slope 0.2)
so with y = up4(xb):
  out = Down(0.4*sqrt2*|y|)  +  Down(0.6*sqrt2*y)
The second (linear) term collapses through the composed matrices
Mv = A@D so it never touches the big upsampled grid.

Stages (matmul contraction is always the SBUF partition dim; the image
data is the stationary lhsT so the kept axis lands on the output
partitions, chaining without transposes):
  s1  v1[w,ho]   = sum_h xb[h,w] A[h,ho]            1 MM  N=512
  sA  u1[w,hd]   = sum_h xb[h,w] Mv[h,hd]           1 MM  N=256   (linear)
  s2  p2[ho,wo]  = sum_w v1[w,ho] A2[w,wo]          4 MM  N=512   (A2=0.4*sqrt2*A)
  abs Y = |p2|                                      (one ACT/DVE pass)
  s3  z[wo,hd]   = sum_ho Y[ho,wo] D[ho,hd]        16 MM  banded N<=70
  s4  o[hd,wd]   = sum_wo z[wo,hd] D[wo,wd]         8 MM  banded N<=70
  sB  o += sum_w u1[w,hd] Mh[w,wd]                  2 MM  N=236   (Mh=0.6*sqrt2*Mv)
All matmul operands are fp16 (1 cycle/row at any N, FWL weight loads);
PSUM accumulation is fp32.

v13 over v3 (449.9us -> 385.0us):
  - s3's four wo-chunk PSUM tiles become two [128,2,256] slot-pairs (one
    bank each, single start=True per bank + hardware lazy zero-on-first-
    write), each drained by ONE batched [128,2,236] cast: 2 fewer Vector
    instructions per image and less PSUM pool coupling.
  - s4/sB's two output row-blocks share one [128,2,256] bank the same
    way, with a single batched output copy: removes the per-image PE
    stall where the mo=1 matmul group waited on the mo=0 output copy
    (ps4 had bufs=1), and saves another Vector instruction.
"""
import numpy as np

UP, DOWN, MARGIN, NEG_SLOPE = 4, 2, 10, 0.2
SQRT2 = 1.4142135623730951
H = W = 128
OUT = 236
NCORES = 8
NIMG = 128

# down-matrix window per 128-row K-chunk: D[s,o] nonzero for s in [2o-5,2o+6]
DWIN = [(0, 67), (61, 131), (125, 195), (189, 256)]

_cache = {}


def _build_nc(nimg=NIMG):
    import concourse.bacc as bacc
    import concourse.bass as bass
    import concourse.tile as tile
    from concourse import mybir

    F32 = mybir.dt.float32
    F16 = mybir.dt.float16
    AF = mybir.ActivationFunctionType
    ALU = mybir.AluOpType

    nc = bacc.Bacc("TRN2", target_bir_lowering=False)
    x_d = nc.dram_tensor("x", [nimg, H, W], F32, kind="ExternalInput")
    b_d = nc.dram_tensor("bias", [nimg], F32, kind="ExternalInput")
    c_d = nc.dram_tensor("cm", [128, 2048], F16, kind="ExternalInput")
    o_d = nc.dram_tensor("out", [nimg, OUT, OUT], F32, kind="ExternalOutput")

    with tile.TileContext(nc) as tc:
        with (
            tc.tile_pool(name="const", bufs=1) as const,
            tc.tile_pool(name="xin", bufs=4) as xin,
            tc.tile_pool(name="xbp", bufs=2) as xbp,
            tc.tile_pool(name="v1p", bufs=2) as v1p,
            tc.tile_pool(name="u1p", bufs=2) as u1p,
            tc.tile_pool(name="yp", bufs=2) as yp,
            tc.tile_pool(name="zp", bufs=2) as zp,
            tc.tile_pool(name="op", bufs=4) as op_,
            tc.tile_pool(name="ps", bufs=2, space="PSUM") as ps,
            tc.tile_pool(name="ps3", bufs=3, space="PSUM") as ps3,
            tc.tile_pool(name="ps4", bufs=1, space="PSUM") as ps4,
        ):
            cm = const.tile([128, 2048], F16)
            nc.sync.dma_start(out=cm, in_=c_d[:])
            A_sb = cm[:, 0:512]
            A2_sb = cm[:, 512:1024]                    # 0.4*sqrt2*A
            dw0 = 1024

            def D_sb(k):
                o0, o1 = DWIN[k]
                return cm[:, dw0 + 70 * k: dw0 + 70 * k + (o1 - o0)]

            mv0 = dw0 + 280                            # Mv [128,256]
            Mv_sb = cm[:, mv0: mv0 + 256]
            mh0 = mv0 + 256                            # Mh [128,236] (cropped)
            Mh_sb = cm[:, mh0: mh0 + 236]

            bb = const.tile([128, nimg], F32)
            nc.gpsimd.dma_start(
                out=bb,
                in_=bass.AP(tensor=b_d[:].tensor, offset=0,
                            ap=[[0, 128], [1, nimg]]),
            )

            # warm PE's clock on the const DMA lane
            pwarm = ps3.tile([128, 256], F32, name="p3")
            nc.tensor.matmul(out=pwarm[:32, :256], lhsT=cm[:, :32],
                             rhs=cm[:, :256], start=True, stop=True)

            for i in range(nimg):
                X = xin.tile([128, W], F32)
                nc.sync.dma_start(out=X, in_=x_d[i])
                Xb = xbp.tile([128, W], F16)
                nc.scalar.activation(out=Xb, in_=X, func=AF.Identity,
                                     bias=bb[:, i:i + 1], scale=1.0)

                # s1: up vertical
                P1 = ps.tile([128, 512], F32, name="p1")
                nc.tensor.matmul(out=P1, lhsT=Xb, rhs=A_sb,
                                 start=True, stop=True)
                V1 = v1p.tile([128, 512], F16)
                nc.vector.tensor_copy(out=V1, in_=P1)

                # sA: linear path, vertical compose
                PA = ps3.tile([128, 256], F32, name="p3")
                nc.tensor.matmul(out=PA, lhsT=Xb, rhs=Mv_sb,
                                 start=True, stop=True)
                U1 = u1p.tile([128, OUT], F16)
                nc.vector.tensor_copy(out=U1, in_=PA[:, MARGIN:MARGIN + OUT])

                # s2 + |.|: up horizontal then one-pass abs evacuation
                Y = yp.tile([128, 4, 512], F16)
                for m in range(4):
                    P2 = ps.tile([128, 512], F32, name="p2")
                    nc.tensor.matmul(out=P2, lhsT=V1[:, 128 * m:128 * (m + 1)],
                                     rhs=A2_sb, start=True, stop=True)
                    nc.scalar.activation(out=Y[:, m, :], in_=P2,
                                         func=AF.Abs, bias=0.0, scale=1.0)

                # s3: down vertical (banded); wo-chunk pairs share a PSUM
                # bank so each pair drains with one batched cast
                Z = zp.tile([128, 4, OUT], F16)
                for half in range(2):
                    P3 = ps3.tile([128, 2, 256], F32, name="p3")
                    for s in range(2):
                        m = 2 * half + s
                        for k in range(4):
                            o0, o1 = DWIN[k]
                            nc.tensor.matmul(
                                out=P3[:, s, o0:o1],
                                lhsT=Y[:, k, 128 * m:128 * (m + 1)],
                                rhs=D_sb(k), start=(s == 0 and k == 0),
                                stop=(k == 3), skip_group_check=True)
                    nc.vector.tensor_copy(
                        out=Z[:, 2 * half:2 * half + 2, :],
                        in_=P3[:, :, MARGIN:MARGIN + OUT])

                # s4 + sB: down horizontal (banded) + linear-path accumulate
                # both row-blocks in one PSUM bank (slot pair, single start)
                P4 = ps4.tile([128, 2, 256], F32, name="p4")
                for mo, (h0, h1) in enumerate(((0, 128), (128, OUT))):
                    rows = h1 - h0
                    for k in range(4):
                        o0, o1 = DWIN[k]
                        nc.tensor.matmul(
                            out=P4[:rows, mo, o0:o1],
                            lhsT=Z[:, k, h0:h1],
                            rhs=D_sb(k), start=(mo == 0 and k == 0),
                            stop=False, skip_group_check=True)
                    # linear path accumulates into the same PSUM group
                    nc.tensor.matmul(
                        out=P4[:rows, mo, MARGIN:MARGIN + OUT],
                        lhsT=U1[:, h0:h1],
                        rhs=Mh_sb, start=False, stop=(mo == 1),
                        skip_group_check=True)
                O = op_.tile([128, 2, OUT], F32)
                nc.vector.tensor_copy(out=O, in_=P4[:, :, MARGIN:MARGIN + OUT])
                nc.sync.dma_start(out=o_d[i, 0:128, :], in_=O[:, 0, :])
                nc.sync.dma_start(out=o_d[i, 128:OUT, :], in_=O[:OUT - 128, 1, :])

    nc.finalize()
    return nc


def _filter_matrices(up_filter, down_filter):
    fu = np.asarray(up_filter, dtype=np.float64)
    fd = np.asarray(down_filter, dtype=np.float64)
    i = np.arange(128)[:, None]
    o = np.arange(512)[None, :]
    t = 10 + o - 4 * i
    A = np.where((t >= 0) & (t < 24), fu[np.clip(t, 0, 23)], 0.0)
    s = np.arange(512)[:, None]
    o2 = np.arange(256)[None, :]
    t2 = 6 + 2 * o2 - s
    D = np.where((t2 >= 0) & (t2 < 12), fd[np.clip(t2, 0, 11)], 0.0)
    return A, D


def _pack_consts(up_filter, down_filter):
    A, D = _filter_matrices(up_filter, down_filter)
    cm = np.zeros((128, 2048), dtype=np.float16)
    cm[:, 0:512] = A.astype(np.float16)
    cm[:, 512:1024] = (A * (0.4 * SQRT2)).astype(np.float16)
    dw0 = 1024
    for k, (o0, o1) in enumerate(DWIN):
        cm[:, dw0 + 70 * k: dw0 + 70 * k + (o1 - o0)] = \
            D[128 * k:128 * (k + 1), o0:o1].astype(np.float16)
    Mv = A @ D
    mv0 = dw0 + 280
    cm[:, mv0: mv0 + 256] = Mv.astype(np.float16)
    mh0 = mv0 + 256
    cm[:, mh0: mh0 + 236] = (Mv * (0.6 * SQRT2))[:, 10:246].astype(np.float16)
    return cm


def _run(x, bias, up_filter, down_filter, trace=False):
    from concourse.bass_utils import run_bass_kernel_spmd

    if "nc" not in _cache:
        _cache["nc"] = _build_nc()
    nc = _cache["nc"]

    cm = _pack_consts(up_filter, down_filter)
    xf = np.ascontiguousarray(np.asarray(x, dtype=np.float32)
                              .reshape(NCORES * NIMG, H, W))
    bias = np.asarray(bias, dtype=np.float32)
    bias_full = np.tile(bias, (NCORES * NIMG) // bias.shape[0])

    in_maps = []
    for c in range(NCORES):
        in_maps.append({
            "x": xf[NIMG * c: NIMG * (c + 1)],
            "bias": np.ascontiguousarray(bias_full[NIMG * c: NIMG * (c + 1)]),
            "cm": cm,
        })
    res = run_bass_kernel_spmd(nc, in_maps, core_ids=list(range(NCORES)),
                               trace=trace)
    out = np.concatenate([res.results[c]["out"][None] for c in range(NCORES)], 0)
    out = out.reshape(4, 256, OUT, OUT)
    return out, res


def kernel(x, bias, up_filter, down_filter):
    out, _ = _run(x, bias, up_filter, down_filter, trace=False)
    return out


def kernel_traced(x, bias, up_filter, down_filter):
    return _run(x, bias, up_filter, down_filter, trace=True)

